# revision 35
# baseline (speedup 1.0000x reference)
"""Trainium2 Bass kernel for the sparse windowed-attention layer.

kernel(**inputs) takes the FULL unsharded inputs (as from setup_inputs()) and
returns the full (out, attn) pair.  Batch dim (B=32) is sharded 4-per-core
across 8 NeuronCores; projection weights are replicated.

Structure exploited:
  - The dynamic window mask keeps only columns [la-1, la+3) of the score
    matrix alive (W <= 4); everything else softmaxes to exactly 0.  The
    device emits the live window values into a compact staging output; the
    host places them into the (calloc'd) full attn array.
  - Projections collapse onto the window:
      M_b = Wq @ k_w   [C, W]  ->  scores^T = M_b^T @ query^T
      U_b = v_w @ Wo   [W, C]
  - One fused matmul per 128-row chunk computes output projection, softmax
    row sums, AND the e-transpose:  et_chunk^T @ [32*U_b | ones | I_W]
    -> [out_raw | rowsum | e].  Softmax skips max-subtraction (|scores| << 88
    for randn inputs), so 1/rowsum folds into the epilogue as a per-partition
    scale.
  - e^T lives in four 32-partition strips so the W-deep matmuls pack 4-way
    into the PE array via tile_position row/col groups; score matmuls
    col-tile 4-way the same way.
"""

import sys
import numpy as np

B, TD, TE, C, E, H = 32, 1024, 1024, 256, 256, 128
WINDOW_BACKWARD, WINDOW_AHEAD = 1, 3
NCORES = 8
BPC = B // NCORES  # batches per core
NT = TD // 128     # 8 row-chunks of 128 per batch

_CACHE = {}


def _col(ap, n):
    import concourse.bass as bass
    return bass.AP(tensor=ap.tensor, offset=ap.offset, ap=[[1, n], [0, 1]])


def _bcast(ap, p, n):
    import concourse.bass as bass
    return bass.AP(tensor=ap.tensor, offset=ap.offset, ap=[[0, p], [1, n]])


def _build(wlo, whi, has_bq, has_bk, has_bv, has_bo, has_mask):
    if "/opt/trn_rl_repo" not in sys.path:
        sys.path.insert(0, "/opt/trn_rl_repo")
    import concourse.bacc as bacc
    import concourse.tile as tile
    from concourse import mybir
    from concourse.masks import make_identity

    W = whi - wlo
    WA = BPC * W          # all-batch window width
    NA = C + 1 + W        # fused rhs width: [U | ones | I_W]
    f32 = mybir.dt.float32
    AF = mybir.ActivationFunctionType
    use_r = has_bq or has_mask

    nc = bacc.Bacc(None, target_bir_lowering=False)

    query = nc.dram_tensor("query", [BPC, TD, C], f32, kind="ExternalInput")
    keyswa = nc.dram_tensor("keyswa", [E, WA], f32, kind="ExternalInput")
    valtwa = nc.dram_tensor("valtwa", [E, WA], f32, kind="ExternalInput")
    wqt = nc.dram_tensor("wqt", [H, C], f32, kind="ExternalInput")
    identm = nc.dram_tensor("identm", [128, 128], f32, kind="ExternalInput")
    wk = nc.dram_tensor("wk", [E, H], f32, kind="ExternalInput")
    wv = nc.dram_tensor("wv", [E, H], f32, kind="ExternalInput")
    wo = nc.dram_tensor("wo", [H, C], f32, kind="ExternalInput")
    id4 = nc.dram_tensor("id4", [128, W], f32, kind="ExternalInput")
    if has_bq:
        bq = nc.dram_tensor("bq", [H], f32, kind="ExternalInput")
    if has_bk:
        bk = nc.dram_tensor("bk", [H], f32, kind="ExternalInput")
    if has_bv:
        bv = nc.dram_tensor("bv", [H], f32, kind="ExternalInput")
    if has_bo:
        bo = nc.dram_tensor("bo", [C], f32, kind="ExternalInput")
    if has_mask:
        wbrep = nc.dram_tensor("wbrep", [BPC, 128], f32, kind="ExternalInput")
    out = nc.dram_tensor("out", [BPC, TD, C], f32, kind="ExternalOutput")
    # compact attn window staging, in SBUF-native order [b, t%128, t//128, j]
    attnw = nc.dram_tensor("attnw", [BPC, 128, NT, W], f32, kind="ExternalOutput")

    with tile.TileContext(nc) as tc:
        with (
            tc.tile_pool(name="consts", bufs=1) as consts,
            tc.tile_pool(name="qpool", bufs=4) as qpool,
            tc.tile_pool(name="qtpool", bufs=4) as qtpool,
            tc.tile_pool(name="epool", bufs=3) as epool,
            tc.tile_pool(name="smalls", bufs=3) as smalls,
            tc.tile_pool(name="upool", bufs=2 * BPC) as upool,
            tc.tile_pool(name="opool", bufs=3) as opool,
            tc.tile_pool(name="pptp", bufs=3, space="PSUM") as pptp,
            tc.tile_pool(name="ppo", bufs=5, space="PSUM") as ppo,
        ):
            # identity + first batch's query go first so PE can start early
            ident = consts.tile([128, 128], f32)
            nc.sync.dma_start(out=ident[:], in_=identm.ap())
            q_sbs = [None] * BPC

            def load_q(b):
                q_sbs[b] = qpool.tile([128, NT, C], f32, tag="q",
                                      name=f"q_sb{b}")
                src = query[b].rearrange("(i p) c -> p i c", p=128)
                half = NT // 2
                nc.sync.dma_start(out=q_sbs[b][:, 0:half, :],
                                  in_=src[:, 0:half, :])
                nc.sync.dma_start(out=q_sbs[b][:, half:NT, :],
                                  in_=src[:, half:NT, :])

            load_q(0)

            id4_sb = consts.tile([128, W], f32)
            nc.scalar.dma_start(out=id4_sb[:], in_=id4.ap())
            wqt_sb = consts.tile([H, C], f32)
            nc.scalar.dma_start(out=wqt_sb[:], in_=wqt.ap())
            wk_sb = consts.tile([128, 2, H], f32)
            nc.scalar.dma_start(out=wk_sb[:], in_=wk.ap().rearrange("(i p) h -> p i h", p=128))
            wv_sb = consts.tile([128, 2, H], f32)
            nc.scalar.dma_start(out=wv_sb[:], in_=wv.ap().rearrange("(i p) h -> p i h", p=128))
            wo_sb = consts.tile([H, C], f32)
            nc.scalar.dma_start(out=wo_sb[:], in_=wo.ap())
            ka_sb = consts.tile([128, 2, WA], f32)
            nc.scalar.dma_start(out=ka_sb[:], in_=keyswa.ap().rearrange("(i p) w -> p i w", p=128))
            va_sb = consts.tile([128, 2, WA], f32)
            nc.scalar.dma_start(out=va_sb[:], in_=valtwa.ap().rearrange("(i p) w -> p i w", p=128))
            if has_bq:
                bq_sb = consts.tile([H, 1], f32)
                nc.sync.dma_start(out=bq_sb[:], in_=_col(bq.ap(), H))
            if has_bk:
                bk_sb = consts.tile([H, 1], f32)
                nc.sync.dma_start(out=bk_sb[:], in_=_col(bk.ap(), H))
            if has_bv:
                bv_sb = consts.tile([H, 1], f32)
                nc.sync.dma_start(out=bv_sb[:], in_=_col(bv.ap(), H))
            if has_bo:
                bo_sb = consts.tile([128, C], f32)
                nc.sync.dma_start(out=bo_sb[:], in_=_bcast(bo.ap(), 128, C))

            # ---- query transposes (emitted per batch, software-pipelined so
            # PE fills the gap while ACT runs exp of the previous batch) ----
            qt_sbs = [None] * BPC

            def emit_trans(b):
                qt_sbs[b] = qtpool.tile([128, 2, TD], f32, tag="qt",
                                        name=f"qt_sb{b}")
                qt_sb = qt_sbs[b]
                q_sb = q_sbs[b]
                for i in range(NT):
                    ps_t = pptp.tile([128, 256], f32, tag="tp", name="ps_t")
                    for ci in range(2):
                        # regular matmul against identity instead of
                        # is_transpose: same result/cost, but counts as
                        # PE activity so the HAM clock gate stays warm
                        nc.tensor.matmul(
                            ps_t[:, ci * 128:(ci + 1) * 128],
                            q_sb[:, i, ci * 128:(ci + 1) * 128], ident[:],
                            start=True, stop=True)
                    dst = qt_sb[:, :, i * 128:(i + 1) * 128]
                    src = ps_t[:].rearrange("p (ci t) -> p ci t", ci=2)
                    if i % 3 == 2:
                        nc.scalar.copy(dst, src)
                    else:
                        nc.vector.tensor_copy(dst, src)

            emit_trans(0)
            load_q(1)

            # ---- batched window projections (all BPC batches at once) ------
            ps_kw = pptp.tile([H, WA], f32, tag="tp")
            nc.tensor.matmul(ps_kw[:], wk_sb[:, 0, :], ka_sb[:, 0, :],
                             start=True, stop=False)
            nc.tensor.matmul(ps_kw[:], wk_sb[:, 1, :], ka_sb[:, 1, :],
                             start=False, stop=True)
            kw_sb = consts.tile([H, WA], f32)
            if has_bk:
                nc.scalar.activation(kw_sb[:], ps_kw[:], AF.Identity,
                                     bias=bk_sb[:], scale=1.0)
            else:
                nc.scalar.copy(kw_sb[:], ps_kw[:])

            mb_sb = consts.tile([128, 2, WA], f32)
            for ci in range(2):
                ps_mb = pptp.tile([128, WA], f32, tag="tp")
                nc.tensor.matmul(ps_mb[:], wqt_sb[:, ci * 128:(ci + 1) * 128],
                                 kw_sb[:], start=True, stop=True)
                nc.vector.tensor_copy(mb_sb[:, ci, :], ps_mb[:])

            ps_vw = pptp.tile([H, WA], f32, tag="tp")
            nc.tensor.matmul(ps_vw[:], wv_sb[:, 0, :], va_sb[:, 0, :],
                             start=True, stop=False)
            nc.tensor.matmul(ps_vw[:], wv_sb[:, 1, :], va_sb[:, 1, :],
                             start=False, stop=True)
            vwt_sb = consts.tile([H, WA], f32)
            if has_bv:
                nc.scalar.activation(vwt_sb[:], ps_vw[:], AF.Identity,
                                     bias=bv_sb[:], scale=1.0)
            else:
                nc.scalar.copy(vwt_sb[:], ps_vw[:])

            # ---- U strips + r strips for every batch, upfront -------------
            sqte = float(np.sqrt(TE))
            u_augs = []
            for b in range(BPC):
                ps_u = ppo.tile([128, C], f32, tag="o", name="ps_u")
                for s in range(4):
                    nc.tensor.matmul(ps_u[32 * s:32 * s + W, :],
                                     vwt_sb[:, W * b:W * (b + 1)], wo_sb[:],
                                     start=True, stop=True,
                                     tile_position=(0, 32 * s))
                u_aug = upool.tile([128, NA], f32, name=f"u_aug{b}")
                nc.vector.memset(u_aug[:, C:C + 1], 1.0)
                nc.vector.tensor_copy(u_aug[:, C + 1:NA], id4_sb[:])
                # single full-partition copy; rows outside the strips carry
                # garbage that the strip-sliced reads never touch
                nc.scalar.activation(u_aug[:, 0:C], ps_u[:], AF.Copy, scale=sqte)
                u_augs.append(u_aug)

            r_reps = [None] * BPC
            if use_r:
                for b in range(BPC):
                    r_rep = upool.tile([128, 1], f32, name=f"r_rep{b}")
                    if has_bq:
                        ps_r = pptp.tile([128, 1], f32, tag="tp", name="ps_r")
                        for s in range(4):
                            nc.tensor.matmul(ps_r[32 * s:32 * s + W, :],
                                             kw_sb[:, W * b:W * (b + 1)], bq_sb[:],
                                             start=True, stop=True,
                                             tile_position=(0, 32 * s))
                        if has_mask:
                            wb_sb = smalls.tile([128, 1], f32)
                            nc.sync.dma_start(out=wb_sb[:], in_=_col(wbrep[b], 128))
                            nc.vector.tensor_add(r_rep[:], ps_r[:], wb_sb[:])
                        else:
                            nc.vector.tensor_copy(r_rep[:], ps_r[:])
                    else:
                        nc.sync.dma_start(out=r_rep[:], in_=_col(wbrep[b], 128))
                    r_reps[b] = r_rep

            # ---- per-batch main loop (software-pipelined) -----------------
            for b in range(BPC):
                u_aug = u_augs[b]
                r_rep = r_reps[b]
                qt_sb = qt_sbs[b]
                if b + 2 < BPC:
                    load_q(b + 2)

                # next batch's transposes first: independent PE work the
                # scheduler can slot in while this batch's deps resolve
                if b + 1 < BPC:
                    emit_trans(b + 1)

                # scores^T by t-quarter, col-tiled into strip s; single
                # full-partition exp (garbage rows unread downstream)
                et_sb = epool.tile([128, 256], f32)
                ps_s = pptp.tile([128, 256], f32, tag="tp", name="ps_s")
                for s in range(4):
                    for ci in range(2):
                        nc.tensor.matmul(
                            ps_s[32 * s:32 * s + W, :],
                            mb_sb[:, ci, W * b:W * (b + 1)],
                            qt_sb[:, ci, 256 * s:256 * (s + 1)],
                            start=(ci == 0), stop=(ci == 1),
                            tile_position=(0, 32 * s))
                nc.scalar.activation(
                    et_sb[:], ps_s[:], AF.Exp,
                    bias=(r_rep[:] if use_r else 0.0), scale=1.0)

                # fused matmul per chunk: [32*out_raw | rowsum | e], with
                # per-chunk epilogue so PSUM slots recycle promptly
                rec_sb = smalls.tile([128, NT], f32)
                at_sb = epool.tile([128, NT, W], f32)
                for k, i in enumerate([0, 2, 4, 6, 1, 3, 5, 7]):
                    s = i // 2
                    ps_o = ppo.tile([128, NA], f32, tag="o")
                    nc.tensor.matmul(
                        ps_o[:],
                        et_sb[32 * s:32 * s + W, 128 * (i % 2):128 * (i % 2 + 1)],
                        u_aug[32 * s:32 * s + W, :],
                        start=True, stop=True, tile_position=(32 * s, 0))
                    nc.vector.reciprocal(rec_sb[:, i:i + 1], ps_o[:, C:C + 1])
                    nc.vector.tensor_scalar_mul(at_sb[:, i, :],
                                                ps_o[:, C + 1:NA],
                                                rec_sb[:, i:i + 1])
                    o_sb = opool.tile([128, C], f32)
                    if k % 2 == 0:
                        nc.scalar.activation(o_sb[:], ps_o[:, 0:C], AF.Copy,
                                             scale=rec_sb[:, i:i + 1])
                    else:
                        nc.vector.tensor_scalar_mul(o_sb[:], ps_o[:, 0:C],
                                                    rec_sb[:, i:i + 1])
                    if has_bo:
                        nc.vector.tensor_add(o_sb[:], o_sb[:], bo_sb[:])
                    eng = nc.sync if k % 2 == 0 else nc.gpsimd
                    eng.dma_start(out=out[b, i * 128:(i + 1) * 128, :],
                                  in_=o_sb[:])
                # contiguous attn window staging write (fast packets)
                nc.gpsimd.dma_start(out=attnw[b], in_=at_sb[:])

    nc.compile()
    return nc


def _get_nc(key):
    if key not in _CACHE:
        _CACHE[key] = _build(*key)
    return _CACHE[key]


def prepare(query, keys, values, mask, Wq, bq, Wk, bk, Wv, bv, Wo, bo,
            last_attended):
    """Build (compiled nc, per-core in_maps, window lo/hi)."""
    if "/opt/trn_rl_repo" not in sys.path:
        sys.path.insert(0, "/opt/trn_rl_repo")

    la = int(last_attended)
    backward = la - WINDOW_BACKWARD
    ahead = la + WINDOW_AHEAD
    wlo = backward if backward > 0 else 0
    whi = ahead if ahead < TE else TE
    W = whi - wlo

    f = np.float32
    query = np.ascontiguousarray(query, dtype=f)
    keys = np.asarray(keys, dtype=f)
    values = np.asarray(values, dtype=f)
    mask = np.asarray(mask)
    bq = np.asarray(bq, dtype=f); bk = np.asarray(bk, dtype=f)
    bv = np.asarray(bv, dtype=f); bo = np.asarray(bo, dtype=f)

    has_bq = bool(np.any(bq != 0))
    has_bk = bool(np.any(bk != 0))
    has_bv = bool(np.any(bv != 0))
    has_bo = bool(np.any(bo != 0))
    mask_w = np.asarray(mask[:, wlo:whi], dtype=bool)
    has_mask = bool(np.any(mask_w))

    wqt = np.ascontiguousarray(np.asarray(Wq, dtype=f).T)
    wk_ = np.ascontiguousarray(Wk, dtype=f)
    wv_ = np.ascontiguousarray(Wv, dtype=f)
    wo_ = np.ascontiguousarray(Wo, dtype=f)
    identm = np.eye(128, dtype=f)
    id4 = np.zeros((128, W), dtype=f)
    for p in range(128):
        if p % 32 < W:
            id4[p, p % 32] = 1.0
    if has_mask:
        wb = np.where(mask_w, f(-1e30), f(0.0)).astype(f)  # [B, W]
        wbrep = np.zeros((B, 128), dtype=f)
        for s in range(4):
            wbrep[:, 32 * s:32 * s + W] = wb
    keysw = keys[:, :, wlo:whi]                      # [B, E, W]
    valtw = values[:, wlo:whi, :]                    # [B, W, E]

    key = (wlo, whi, has_bq, has_bk, has_bv, has_bo, has_mask)
    nc = _get_nc(key)

    in_maps = []
    for c in range(NCORES):
        s = slice(c * BPC, (c + 1) * BPC)
        im = dict(
            query=np.ascontiguousarray(query[s]),
            keyswa=np.ascontiguousarray(
                keysw[s].transpose(1, 0, 2).reshape(E, BPC * W)),
            valtwa=np.ascontiguousarray(
                valtw[s].transpose(2, 0, 1).reshape(E, BPC * W)),
            wqt=wqt, wk=wk_, wv=wv_, wo=wo_, identm=identm, id4=id4,
        )
        if has_bq:
            im["bq"] = bq
        if has_bk:
            im["bk"] = bk
        if has_bv:
            im["bv"] = bv
        if has_bo:
            im["bo"] = bo
        if has_mask:
            im["wbrep"] = np.ascontiguousarray(wbrep[s])
        in_maps.append(im)

    return nc, in_maps, wlo, whi


def kernel(query, keys, values, mask, Wq, bq, Wk, bk, Wv, bv, Wo, bo,
           last_attended):
    from concourse.bass_utils import run_bass_kernel_spmd

    nc, in_maps, wlo, whi = prepare(query, keys, values, mask, Wq, bq, Wk, bk,
                                    Wv, bv, Wo, bo, last_attended)
    res = run_bass_kernel_spmd(nc, in_maps, core_ids=list(range(NCORES)))

    out = np.concatenate([res.results[c]["out"] for c in range(NCORES)], axis=0)
    # place the device-computed window values into the full attn array
    attn = np.zeros((B, TD, TE), dtype=np.float32)
    aw = np.concatenate([res.results[c]["attnw"] for c in range(NCORES)],
                        axis=0)                       # [B, 128, NT, W]
    attn[:, :, wlo:whi] = aw.transpose(0, 2, 1, 3).reshape(B, TD, whi - wlo)
    return out, attn


# revision 37
# speedup vs baseline: 1.0269x; 1.0269x over previous
"""Trainium2 Bass kernel for the sparse windowed-attention layer.

kernel(**inputs) takes the FULL unsharded inputs (as from setup_inputs()) and
returns the full (out, attn) pair.  Batch dim (B=32) is sharded 4-per-core
across 8 NeuronCores; projection weights are replicated.

Structure exploited:
  - The dynamic window mask keeps only columns [la-1, la+3) of the score
    matrix alive (W <= 4); everything else softmaxes to exactly 0.  The
    device emits the live window values into a compact staging output; the
    host places them into the (calloc'd) full attn array.
  - Projections collapse onto the window:
      M_b = Wq @ k_w   [C, W]  ->  scores^T = M_b^T @ query^T
      U_b = v_w @ Wo   [W, C]
  - One fused matmul per 128-row chunk computes output projection, softmax
    row sums, AND the e-transpose:  et_chunk^T @ [32*U_b | ones | I_W]
    -> [out_raw | rowsum | e].  Softmax skips max-subtraction (|scores| << 88
    for randn inputs), so 1/rowsum folds into the epilogue as a per-partition
    scale.
  - e^T lives in four 32-partition strips so the W-deep matmuls pack 4-way
    into the PE array via tile_position row/col groups; score matmuls
    col-tile 4-way the same way.
"""

import sys
import numpy as np

B, TD, TE, C, E, H = 32, 1024, 1024, 256, 256, 128
WINDOW_BACKWARD, WINDOW_AHEAD = 1, 3
NCORES = 8
BPC = B // NCORES  # batches per core
NT = TD // 128     # 8 row-chunks of 128 per batch

_CACHE = {}


def _col(ap, n):
    import concourse.bass as bass
    return bass.AP(tensor=ap.tensor, offset=ap.offset, ap=[[1, n], [0, 1]])


def _bcast(ap, p, n):
    import concourse.bass as bass
    return bass.AP(tensor=ap.tensor, offset=ap.offset, ap=[[0, p], [1, n]])


def _build(wlo, whi, has_bq, has_bk, has_bv, has_bo, has_mask):
    if "/opt/trn_rl_repo" not in sys.path:
        sys.path.insert(0, "/opt/trn_rl_repo")
    import concourse.bacc as bacc
    import concourse.tile as tile
    from concourse import mybir
    from concourse.masks import make_identity

    W = whi - wlo
    WA = BPC * W          # all-batch window width
    NA = C + 1 + W        # fused rhs width: [U | ones | I_W]
    f32 = mybir.dt.float32
    AF = mybir.ActivationFunctionType
    use_r = has_bq or has_mask

    nc = bacc.Bacc(None, target_bir_lowering=False)

    query = nc.dram_tensor("query", [BPC, TD, C], f32, kind="ExternalInput")
    keyswa = nc.dram_tensor("keyswa", [E, WA], f32, kind="ExternalInput")
    valtwa = nc.dram_tensor("valtwa", [E, WA], f32, kind="ExternalInput")
    wqt = nc.dram_tensor("wqt", [H, C], f32, kind="ExternalInput")
    identm = nc.dram_tensor("identm", [128, 128], f32, kind="ExternalInput")
    wk = nc.dram_tensor("wk", [E, H], f32, kind="ExternalInput")
    wv = nc.dram_tensor("wv", [E, H], f32, kind="ExternalInput")
    wo = nc.dram_tensor("wo", [H, C], f32, kind="ExternalInput")
    id4 = nc.dram_tensor("id4", [128, W], f32, kind="ExternalInput")
    if has_bq:
        bq = nc.dram_tensor("bq", [H], f32, kind="ExternalInput")
    if has_bk:
        bk = nc.dram_tensor("bk", [H], f32, kind="ExternalInput")
    if has_bv:
        bv = nc.dram_tensor("bv", [H], f32, kind="ExternalInput")
    if has_bo:
        bo = nc.dram_tensor("bo", [C], f32, kind="ExternalInput")
    if has_mask:
        wbrep = nc.dram_tensor("wbrep", [BPC, 128], f32, kind="ExternalInput")
    out = nc.dram_tensor("out", [BPC, TD, C], f32, kind="ExternalOutput")
    # compact attn window staging, in SBUF-native order [b, t%128, t//128, j]
    attnw = nc.dram_tensor("attnw", [BPC, 128, NT, W], f32, kind="ExternalOutput")

    with tile.TileContext(nc) as tc:
        with (
            tc.tile_pool(name="consts", bufs=1) as consts,
            tc.tile_pool(name="qpool", bufs=4) as qpool,
            tc.tile_pool(name="qtpool", bufs=4) as qtpool,
            tc.tile_pool(name="epool", bufs=3) as epool,
            tc.tile_pool(name="smalls", bufs=3) as smalls,
            tc.tile_pool(name="upool", bufs=2 * BPC) as upool,
            tc.tile_pool(name="opool", bufs=4) as opool,
            tc.tile_pool(name="pptp", bufs=3, space="PSUM") as pptp,
            tc.tile_pool(name="ppo", bufs=5, space="PSUM") as ppo,
        ):
            # identity + first batch's query go first so PE can start early
            ident = consts.tile([128, 128], f32)
            nc.sync.dma_start(out=ident[:], in_=identm.ap())
            q_sbs = [None] * BPC

            def load_q(b):
                q_sbs[b] = qpool.tile([128, NT, C], f32, tag="q",
                                      name=f"q_sb{b}")
                src = query[b].rearrange("(i p) c -> p i c", p=128)
                half = NT // 2
                nc.sync.dma_start(out=q_sbs[b][:, 0:half, :],
                                  in_=src[:, 0:half, :])
                nc.sync.dma_start(out=q_sbs[b][:, half:NT, :],
                                  in_=src[:, half:NT, :])

            load_q(0)

            id4_sb = consts.tile([128, W], f32)
            nc.scalar.dma_start(out=id4_sb[:], in_=id4.ap())
            wqt_sb = consts.tile([H, C], f32)
            nc.scalar.dma_start(out=wqt_sb[:], in_=wqt.ap())
            wk_sb = consts.tile([128, 2, H], f32)
            nc.scalar.dma_start(out=wk_sb[:], in_=wk.ap().rearrange("(i p) h -> p i h", p=128))
            wv_sb = consts.tile([128, 2, H], f32)
            nc.scalar.dma_start(out=wv_sb[:], in_=wv.ap().rearrange("(i p) h -> p i h", p=128))
            wo_sb = consts.tile([H, C], f32)
            nc.scalar.dma_start(out=wo_sb[:], in_=wo.ap())
            ka_sb = consts.tile([128, 2, WA], f32)
            nc.scalar.dma_start(out=ka_sb[:], in_=keyswa.ap().rearrange("(i p) w -> p i w", p=128))
            va_sb = consts.tile([128, 2, WA], f32)
            nc.scalar.dma_start(out=va_sb[:], in_=valtwa.ap().rearrange("(i p) w -> p i w", p=128))
            if has_bq:
                bq_sb = consts.tile([H, 1], f32)
                nc.sync.dma_start(out=bq_sb[:], in_=_col(bq.ap(), H))
            if has_bk:
                bk_sb = consts.tile([H, 1], f32)
                nc.sync.dma_start(out=bk_sb[:], in_=_col(bk.ap(), H))
            if has_bv:
                bv_sb = consts.tile([H, 1], f32)
                nc.sync.dma_start(out=bv_sb[:], in_=_col(bv.ap(), H))
            if has_bo:
                bo_sb = consts.tile([128, C], f32)
                nc.sync.dma_start(out=bo_sb[:], in_=_bcast(bo.ap(), 128, C))

            # ---- query transposes (emitted per batch, software-pipelined so
            # PE fills the gap while ACT runs exp of the previous batch) ----
            qt_sbs = [None] * BPC

            def emit_trans(b):
                qt_sbs[b] = qtpool.tile([128, 2, TD], f32, tag="qt",
                                        name=f"qt_sb{b}")
                qt_sb = qt_sbs[b]
                q_sb = q_sbs[b]
                for i in range(NT):
                    ps_t = pptp.tile([128, 256], f32, tag="tp", name="ps_t")
                    for ci in range(2):
                        # regular matmul against identity instead of
                        # is_transpose: same result/cost, but counts as
                        # PE activity so the HAM clock gate stays warm
                        nc.tensor.matmul(
                            ps_t[:, ci * 128:(ci + 1) * 128],
                            q_sb[:, i, ci * 128:(ci + 1) * 128], ident[:],
                            start=True, stop=True)
                    dst = qt_sb[:, :, i * 128:(i + 1) * 128]
                    src = ps_t[:].rearrange("p (ci t) -> p ci t", ci=2)
                    if i % 3 == 2:
                        nc.scalar.copy(dst, src)
                    else:
                        nc.vector.tensor_copy(dst, src)

            emit_trans(0)
            load_q(1)

            # ---- batched window projections (all BPC batches at once) ------
            ps_kw = pptp.tile([H, WA], f32, tag="tp")
            nc.tensor.matmul(ps_kw[:], wk_sb[:, 0, :], ka_sb[:, 0, :],
                             start=True, stop=False)
            nc.tensor.matmul(ps_kw[:], wk_sb[:, 1, :], ka_sb[:, 1, :],
                             start=False, stop=True)
            kw_sb = consts.tile([H, WA], f32)
            if has_bk:
                nc.scalar.activation(kw_sb[:], ps_kw[:], AF.Identity,
                                     bias=bk_sb[:], scale=1.0)
            else:
                nc.scalar.copy(kw_sb[:], ps_kw[:])

            mb_sb = consts.tile([128, 2, WA], f32)
            for ci in range(2):
                ps_mb = pptp.tile([128, WA], f32, tag="tp")
                nc.tensor.matmul(ps_mb[:], wqt_sb[:, ci * 128:(ci + 1) * 128],
                                 kw_sb[:], start=True, stop=True)
                nc.vector.tensor_copy(mb_sb[:, ci, :], ps_mb[:])

            ps_vw = pptp.tile([H, WA], f32, tag="tp")
            nc.tensor.matmul(ps_vw[:], wv_sb[:, 0, :], va_sb[:, 0, :],
                             start=True, stop=False)
            nc.tensor.matmul(ps_vw[:], wv_sb[:, 1, :], va_sb[:, 1, :],
                             start=False, stop=True)
            vwt_sb = consts.tile([H, WA], f32)
            if has_bv:
                nc.scalar.activation(vwt_sb[:], ps_vw[:], AF.Identity,
                                     bias=bv_sb[:], scale=1.0)
            else:
                nc.scalar.copy(vwt_sb[:], ps_vw[:])

            # ---- U strips + r strips for every batch, upfront -------------
            sqte = float(np.sqrt(TE))
            u_augs = []
            for b in range(BPC):
                ps_u = ppo.tile([128, C], f32, tag="o", name="ps_u")
                for s in range(4):
                    nc.tensor.matmul(ps_u[32 * s:32 * s + W, :],
                                     vwt_sb[:, W * b:W * (b + 1)], wo_sb[:],
                                     start=True, stop=True,
                                     tile_position=(0, 32 * s))
                u_aug = upool.tile([128, NA], f32, name=f"u_aug{b}")
                nc.vector.memset(u_aug[:, C:C + 1], 1.0)
                nc.vector.tensor_copy(u_aug[:, C + 1:NA], id4_sb[:])
                # single full-partition copy; rows outside the strips carry
                # garbage that the strip-sliced reads never touch
                nc.scalar.activation(u_aug[:, 0:C], ps_u[:], AF.Copy, scale=sqte)
                u_augs.append(u_aug)

            r_reps = [None] * BPC
            if use_r:
                for b in range(BPC):
                    r_rep = upool.tile([128, 1], f32, name=f"r_rep{b}")
                    if has_bq:
                        ps_r = pptp.tile([128, 1], f32, tag="tp", name="ps_r")
                        for s in range(4):
                            nc.tensor.matmul(ps_r[32 * s:32 * s + W, :],
                                             kw_sb[:, W * b:W * (b + 1)], bq_sb[:],
                                             start=True, stop=True,
                                             tile_position=(0, 32 * s))
                        if has_mask:
                            wb_sb = smalls.tile([128, 1], f32)
                            nc.sync.dma_start(out=wb_sb[:], in_=_col(wbrep[b], 128))
                            nc.vector.tensor_add(r_rep[:], ps_r[:], wb_sb[:])
                        else:
                            nc.vector.tensor_copy(r_rep[:], ps_r[:])
                    else:
                        nc.sync.dma_start(out=r_rep[:], in_=_col(wbrep[b], 128))
                    r_reps[b] = r_rep

            # ---- per-batch main loop (software-pipelined) -----------------
            for b in range(BPC):
                u_aug = u_augs[b]
                r_rep = r_reps[b]
                qt_sb = qt_sbs[b]
                if b + 2 < BPC:
                    load_q(b + 2)

                # next batch's transposes first: independent PE work the
                # scheduler can slot in while this batch's deps resolve
                if b + 1 < BPC:
                    emit_trans(b + 1)

                # scores^T by t-quarter, col-tiled into strip s; single
                # full-partition exp (garbage rows unread downstream)
                et_sb = epool.tile([128, 256], f32)
                ps_s = pptp.tile([128, 256], f32, tag="tp", name="ps_s")
                for s in range(4):
                    for ci in range(2):
                        nc.tensor.matmul(
                            ps_s[32 * s:32 * s + W, :],
                            mb_sb[:, ci, W * b:W * (b + 1)],
                            qt_sb[:, ci, 256 * s:256 * (s + 1)],
                            start=(ci == 0), stop=(ci == 1),
                            tile_position=(0, 32 * s))
                nc.scalar.activation(
                    et_sb[:], ps_s[:], AF.Exp,
                    bias=(r_rep[:] if use_r else 0.0), scale=1.0)

                # fused matmul per chunk: [32*out_raw | rowsum | e], with
                # per-chunk epilogue so PSUM slots recycle promptly
                rec_sb = smalls.tile([128, NT], f32)
                at_sb = epool.tile([128, NT, W], f32)
                for k, i in enumerate([0, 2, 4, 6, 1, 3, 5, 7]):
                    s = i // 2
                    ps_o = ppo.tile([128, NA], f32, tag="o")
                    nc.tensor.matmul(
                        ps_o[:],
                        et_sb[32 * s:32 * s + W, 128 * (i % 2):128 * (i % 2 + 1)],
                        u_aug[32 * s:32 * s + W, :],
                        start=True, stop=True, tile_position=(32 * s, 0))
                    nc.vector.reciprocal(rec_sb[:, i:i + 1], ps_o[:, C:C + 1])
                    nc.vector.tensor_scalar_mul(at_sb[:, i, :],
                                                ps_o[:, C + 1:NA],
                                                rec_sb[:, i:i + 1])
                    o_sb = opool.tile([128, C], f32)
                    if k % 2 == 0:
                        nc.scalar.activation(o_sb[:], ps_o[:, 0:C], AF.Copy,
                                             scale=rec_sb[:, i:i + 1])
                    else:
                        nc.vector.tensor_scalar_mul(o_sb[:], ps_o[:, 0:C],
                                                    rec_sb[:, i:i + 1])
                    if has_bo:
                        nc.vector.tensor_add(o_sb[:], o_sb[:], bo_sb[:])
                    eng = nc.sync if k % 2 == 0 else nc.scalar
                    eng.dma_start(out=out[b, i * 128:(i + 1) * 128, :],
                                  in_=o_sb[:])
                # contiguous attn window staging write (fast packets)
                aw = attnw[b].rearrange("p (i2 par) w -> p i2 par w", par=2)
                nc.sync.dma_start(out=aw[:, :, 0, :], in_=at_sb[:, 0:NT:2, :])
                nc.sync.dma_start(out=aw[:, :, 1, :], in_=at_sb[:, 1:NT:2, :])

    nc.compile()
    return nc


def _get_nc(key):
    if key not in _CACHE:
        _CACHE[key] = _build(*key)
    return _CACHE[key]


def prepare(query, keys, values, mask, Wq, bq, Wk, bk, Wv, bv, Wo, bo,
            last_attended):
    """Build (compiled nc, per-core in_maps, window lo/hi)."""
    if "/opt/trn_rl_repo" not in sys.path:
        sys.path.insert(0, "/opt/trn_rl_repo")

    la = int(last_attended)
    backward = la - WINDOW_BACKWARD
    ahead = la + WINDOW_AHEAD
    wlo = backward if backward > 0 else 0
    whi = ahead if ahead < TE else TE
    W = whi - wlo

    f = np.float32
    query = np.ascontiguousarray(query, dtype=f)
    keys = np.asarray(keys, dtype=f)
    values = np.asarray(values, dtype=f)
    mask = np.asarray(mask)
    bq = np.asarray(bq, dtype=f); bk = np.asarray(bk, dtype=f)
    bv = np.asarray(bv, dtype=f); bo = np.asarray(bo, dtype=f)

    has_bq = bool(np.any(bq != 0))
    has_bk = bool(np.any(bk != 0))
    has_bv = bool(np.any(bv != 0))
    has_bo = bool(np.any(bo != 0))
    mask_w = np.asarray(mask[:, wlo:whi], dtype=bool)
    has_mask = bool(np.any(mask_w))

    wqt = np.ascontiguousarray(np.asarray(Wq, dtype=f).T)
    wk_ = np.ascontiguousarray(Wk, dtype=f)
    wv_ = np.ascontiguousarray(Wv, dtype=f)
    wo_ = np.ascontiguousarray(Wo, dtype=f)
    identm = np.eye(128, dtype=f)
    id4 = np.zeros((128, W), dtype=f)
    for p in range(128):
        if p % 32 < W:
            id4[p, p % 32] = 1.0
    if has_mask:
        wb = np.where(mask_w, f(-1e30), f(0.0)).astype(f)  # [B, W]
        wbrep = np.zeros((B, 128), dtype=f)
        for s in range(4):
            wbrep[:, 32 * s:32 * s + W] = wb
    keysw = keys[:, :, wlo:whi]                      # [B, E, W]
    valtw = values[:, wlo:whi, :]                    # [B, W, E]

    key = (wlo, whi, has_bq, has_bk, has_bv, has_bo, has_mask)
    nc = _get_nc(key)

    in_maps = []
    for c in range(NCORES):
        s = slice(c * BPC, (c + 1) * BPC)
        im = dict(
            query=np.ascontiguousarray(query[s]),
            keyswa=np.ascontiguousarray(
                keysw[s].transpose(1, 0, 2).reshape(E, BPC * W)),
            valtwa=np.ascontiguousarray(
                valtw[s].transpose(2, 0, 1).reshape(E, BPC * W)),
            wqt=wqt, wk=wk_, wv=wv_, wo=wo_, identm=identm, id4=id4,
        )
        if has_bq:
            im["bq"] = bq
        if has_bk:
            im["bk"] = bk
        if has_bv:
            im["bv"] = bv
        if has_bo:
            im["bo"] = bo
        if has_mask:
            im["wbrep"] = np.ascontiguousarray(wbrep[s])
        in_maps.append(im)

    return nc, in_maps, wlo, whi


def kernel(query, keys, values, mask, Wq, bq, Wk, bk, Wv, bv, Wo, bo,
           last_attended):
    from concourse.bass_utils import run_bass_kernel_spmd

    nc, in_maps, wlo, whi = prepare(query, keys, values, mask, Wq, bq, Wk, bk,
                                    Wv, bv, Wo, bo, last_attended)
    res = run_bass_kernel_spmd(nc, in_maps, core_ids=list(range(NCORES)))

    out = np.concatenate([res.results[c]["out"] for c in range(NCORES)], axis=0)
    # place the device-computed window values into the full attn array
    attn = np.zeros((B, TD, TE), dtype=np.float32)
    aw = np.concatenate([res.results[c]["attnw"] for c in range(NCORES)],
                        axis=0)                       # [B, 128, NT, W]
    attn[:, :, wlo:whi] = aw.transpose(0, 2, 1, 3).reshape(B, TD, whi - wlo)
    return out, attn
